# revision 12
# baseline (speedup 1.0000x reference)
"""Multi-head attention forward (B=16, S=1024, d=1024, H=16, Dh=64) on 8
Trainium2 NeuronCores, data-parallel over batch (2 batches per core).

v2: restructured for steady-state Tensor-engine cadence (~216ns/matmul,
the 512-col streaming floor at 2.4GHz).
  - Heads processed one at a time; PV chains trail scores by 2 score-tiles
    so the PE never waits on the Scalar-engine exp.
  - K stationaries zero-padded to k=128 (two banded tiles per head pair):
    all matmuls share one PE row geometry, avoiding the ~90ns
    geometry-change penalty on LDWEIGHTS.
  - PSUM: scores 2x[128,1024] (one big exp per tile), PV 2 banks
    (sequential per-(head,chunk) chains), projections 2 banks.
  - All projection work (Q/K/V/out) emitted as uniform 8-matmul chain
    "filler" units interleaved into the attention loop so the Tensor queue
    always has ready work while exp catches up; output projection of the
    last batch drains in waves of 6 chains on borrowed idle PSUM banks.
  - DMA ordered so batch-0 tokens + Wv halves land first (V-proj starts
    ~11us in); Wo lands last (first needed ~280us in).

Device kernel (per core, bf16 matmuls, fp32 accumulate):
  inputs (host-prepped): XT [d, 2048] = hidden[2c:2c+2].reshape(2048,d).T,
  WqT/WkT/WvT = W.T [in, out], WoT = Wo.T [dv, o]  (all bf16),
  bq, bk [1024] f32, bo2 = bo + Wo @ bv  (bv folded: softmax rows sum to 1).

  QT[dq,t] = WqT.T @ XT (+bq)         KT likewise
  V[t,dv]  = XT.T @ WvT               (stored head-split with a ones column)
  per (batch, head):
    scoresT[s,t] = K_h @ Q_h.T        (k=dh=64 contraction, row-group h%2)
    PT[s,t] = exp(scoresT * 0.125)    (no max-subtract: scores bounded)
    ctxT_aug[dv+1,t] = [V_h | 1].T @ PT   (row dv = softmax denominator)
    ctxT_h = ctxT_aug[:dv] * bcast(1/denominator)
  outT[o,t] = WoT.T @ ctxT (+bo2)  ->  host transposes back.
"""

import numpy as np
import ml_dtypes

import concourse.bass as bass
import concourse.mybir as mybir
import concourse.tile as tile
from concourse import bacc
from concourse.bass_utils import run_bass_kernel_spmd

P = 128
D = 1024
T = 2048  # tokens per core
TB = 1024  # tokens per batch (= S)
H = 16
DH = 64
KD = D // P  # 8 partition-tiles of the d/dv/s dims
NB = T // TB  # batches per core
NCORES = 8

BF16 = mybir.dt.bfloat16
F32 = mybir.dt.float32
EXPF = mybir.ActivationFunctionType.Exp
MULT = mybir.AluOpType.mult

# test.py hooks
TRACE = False
TRACE_KWARGS = {}
LAST_RESULTS = None

_NC_CACHE = None


def build_nc():
    nc = bacc.Bacc("TRN2", target_bir_lowering=False, debug=False, num_devices=NCORES)

    xt_d = nc.dram_tensor("xt", [D, T], BF16, kind="ExternalInput")
    wqt_d = nc.dram_tensor("wqt", [D, D], BF16, kind="ExternalInput")
    wkt_d = nc.dram_tensor("wkt", [D, D], BF16, kind="ExternalInput")
    wvt_d = nc.dram_tensor("wvt", [D, D], BF16, kind="ExternalInput")
    wot_d = nc.dram_tensor("wot", [D, D], BF16, kind="ExternalInput")
    bq_d = nc.dram_tensor("bq", [D], F32, kind="ExternalInput")
    bk_d = nc.dram_tensor("bk", [D], F32, kind="ExternalInput")
    bo2_d = nc.dram_tensor("bo2", [D], F32, kind="ExternalInput")
    outt_d = nc.dram_tensor("outt", [D, T], F32, kind="ExternalOutput")

    with tile.TileContext(nc) as tc:
        from contextlib import ExitStack

        with ExitStack() as ctx:
            wpool = ctx.enter_context(tc.tile_pool(name="w", bufs=1))
            xpool = ctx.enter_context(tc.tile_pool(name="x", bufs=1))
            qkpool = ctx.enter_context(tc.tile_pool(name="qk", bufs=2))
            vpool = ctx.enter_context(tc.tile_pool(name="v", bufs=2))
            ptpool = ctx.enter_context(tc.tile_pool(name="pt", bufs=2))
            cpool = ctx.enter_context(tc.tile_pool(name="ctx", bufs=1))
            spool = ctx.enter_context(tc.tile_pool(name="small", bufs=1))
            npool = ctx.enter_context(tc.tile_pool(name="norm", bufs=2))
            opool = ctx.enter_context(tc.tile_pool(name="out", bufs=6))
            scpool = ctx.enter_context(tc.tile_pool(name="sc", bufs=2, space="PSUM"))
            pvpool = ctx.enter_context(tc.tile_pool(name="pv", bufs=2, space="PSUM"))
            fpool = ctx.enter_context(tc.tile_pool(name="fp", bufs=2, space="PSUM"))

            # ---- global loads (ordered so V-proj can start earliest) ----
            xt = [xpool.tile([P, T], BF16, tag=f"xt{k}", name=f"xt{k}") for k in range(KD)]
            wq, wk, wv, wo = (
                [wpool.tile([P, D], BF16, tag=f"w{nm}{k}", name=f"w{nm}{k}") for k in range(KD)]
                for nm in "qkvo"
            )
            bq_sb = spool.tile([P, KD], F32, tag="bq", name="bq_sb")
            bk_sb = spool.tile([P, KD], F32, tag="bk", name="bk_sb")
            bo_sb = spool.tile([P, KD], F32, tag="bo", name="bo_sb")
            for sb, dte in ((bq_sb, bq_d), (bk_sb, bk_d), (bo_sb, bo2_d)):
                nc.sync.dma_start(sb[:], dte.rearrange("(o p) -> p o", p=P))
            # batch-0 tokens + Wv first (V-proj of batch 0 is the first
            # compute), then Wq/Wk (Q/K proj), batch-1 tokens, Wo last.
            for k in range(KD):
                nc.sync.dma_start(xt[k][:, 0:512], xt_d[k * P : (k + 1) * P, 0:512])
                nc.sync.dma_start(wv[k][:, 0:512], wvt_d[k * P : (k + 1) * P, 0:512])
            for k in range(KD):
                nc.sync.dma_start(xt[k][:, 512:TB], xt_d[k * P : (k + 1) * P, 512:TB])
            for k in range(KD):
                nc.sync.dma_start(wv[k][:, 512:D], wvt_d[k * P : (k + 1) * P, 512:D])
            for k in range(KD):
                nc.sync.dma_start(wq[k][:], wqt_d[k * P : (k + 1) * P, :])
                nc.sync.dma_start(wk[k][:], wkt_d[k * P : (k + 1) * P, :])
            for k in range(KD):
                nc.sync.dma_start(xt[k][:, TB:T], xt_d[k * P : (k + 1) * P, TB:T])
            for k in range(KD):
                nc.sync.dma_start(wo[k][:], wot_d[k * P : (k + 1) * P, :])

            # ---- state ----
            tail_prefix = []
            qk_instance = [0]
            v_tiles = {}  # b -> [v tile per mt]
            qk_cur = {}  # j-slot -> (qtj, ktj); only current+next alive
            ctxt_b = {}  # b -> [ctxt tile per m]

            # =========== filler units (uniform 8-matmul chains) ============
            def v_unit(b, mt, c):
                """V[t-tile mt, dv chunk c] for batch b -> v_tiles[b][mt]."""
                vt = v_tiles[b][mt]
                ps = fpool.tile([P, 512], F32, tag="fp", name="fpv")
                for k in range(KD):
                    yield nc.tensor.matmul(
                        ps[:],
                        xt[k][:, (b * KD + mt) * P : (b * KD + mt + 1) * P],
                        wv[k][:, c * 512 : (c + 1) * 512],
                        start=(k == 0),
                        stop=(k == KD - 1),
                    )
                nc.vector.tensor_copy(
                    vt[:, c * 8 : (c + 1) * 8, 0:DH],
                    ps.rearrange("p (h d) -> p h d", d=DH),
                )

            def qk_unit(wt, bias_sb, dests, b, j, c, ps=None):
                """One 512-token chunk of the Q or K projection for head-pair
                j.  dests: list of (tile, p_lo, p_hi) partition-row spans."""
                if ps is None:
                    ps = fpool.tile([P, 512], F32, tag="fp", name="fpq")
                for k in range(KD):
                    yield nc.tensor.matmul(
                        ps[:],
                        wt[k][:, j * P : (j + 1) * P],
                        xt[k][:, b * TB + c * 512 : b * TB + (c + 1) * 512],
                        start=(k == 0),
                        stop=(k == KD - 1),
                    )
                for dest, lo, hi in dests:
                    nc.vector.tensor_scalar_add(
                        dest[lo:hi, c * 512 : (c + 1) * 512],
                        ps[lo:hi, :],
                        bias_sb[lo:hi, j : j + 1],
                    )

            def out_unit(b, mo, c, ps=None, drain="dve"):
                """outT[o-tile mo, chunk c] for batch b."""
                ct = ctxt_b[b]
                if ps is None:
                    ps = fpool.tile([P, 512], F32, tag="fp", name="fpo")
                for k in range(KD):
                    yield nc.tensor.matmul(
                        ps[:],
                        wo[k][:, mo * P : (mo + 1) * P],
                        ct[k][:, c * 512 : (c + 1) * 512],
                        start=(k == 0),
                        stop=(k == KD - 1),
                    )
                osb = opool.tile([P, 512], F32, tag="osb", name="osb")
                if drain == "act":
                    nc.scalar.add(osb[:], ps[:], bo_sb[:, mo : mo + 1])
                else:
                    nc.vector.tensor_scalar_add(osb[:], ps[:], bo_sb[:, mo : mo + 1])
                nc.sync.dma_start(
                    outt_d[
                        mo * P : (mo + 1) * P,
                        b * TB + c * 512 : b * TB + (c + 1) * 512,
                    ],
                    osb[:],
                )

            filler_q = []  # list of active generators

            def pull_fillers(n):
                while n > 0 and filler_q:
                    g = filler_q[0]
                    try:
                        next(g)
                        n -= 1
                    except StopIteration:
                        filler_q.pop(0)

            def drain_fillers():
                while filler_q:
                    pull_fillers(1)

            def make_v_tiles(b):
                tiles = []
                for mt in range(KD):
                    vt = vpool.tile(
                        [P, H, DH + 1], BF16, tag=f"v{mt}", name=f"v{mt}"
                    )
                    nc.vector.memset(vt[:, :, DH : DH + 1], 1.0)
                    tiles.append(vt)
                v_tiles[b] = tiles

            def make_qk(b, j, ps_slots=None):
                # K is stored as two zero-padded [128, TB] tiles (head h in
                # its own 64-row band, zeros elsewhere) so score matmuls run
                # with k=128 stationary geometry — avoiding the ~90ns PE
                # row-geometry-change penalty of k=64 loads.
                qtj = qkpool.tile([P, TB], BF16, tag="qtj", name="qtj")
                ktj0 = qkpool.tile([P, TB], BF16, tag="ktj0", name="ktj0")
                ktj1 = qkpool.tile([P, TB], BF16, tag="ktj1", name="ktj1")
                # zero bands only for the first instance in each rotating
                # buffer: later instances reuse the same SBUF bytes and only
                # rows 0:DH / DH:P are ever rewritten, so zeros persist.
                if qk_instance[0] < 2:
                    nc.vector.memset(ktj0[DH:P, :], 0.0)
                    nc.vector.memset(ktj1[0:DH, :], 0.0)
                qk_instance[0] += 1
                qk_cur[(b, j)] = (qtj, ktj0, ktj1)
                units = []
                slots = ps_slots or [None] * 4
                for c in range(2):
                    units.append(
                        qk_unit(wq, bq_sb, [(qtj, 0, P)], b, j, c, ps=slots[c])
                    )
                for c in range(2):
                    units.append(
                        qk_unit(
                            wk,
                            bk_sb,
                            [(ktj0, 0, DH), (ktj1, DH, P)],
                            b,
                            j,
                            c,
                            ps=slots[2 + c],
                        )
                    )
                return units

            def make_ctxt(b):
                if b == 0:
                    tiles = [
                        cpool.tile([P, TB], BF16, tag=f"ctxt{m}", name=f"ctxt{m}")
                        for m in range(KD)
                    ]
                else:
                    # reuse wv's SBUF slots: wv is fully consumed by the time
                    # batch-1 context tiles are first written (shapes match)
                    tiles = [
                        wpool.tile([P, TB], BF16, tag=f"wv{m}", name=f"ctxb1_{m}")
                        for m in range(KD)
                    ]
                ctxt_b[b] = tiles

            # ================= attention head phase =================
            def normalize(b, h, c, pvt):
                j, row0 = h // 2, (h % 2) * DH
                ct = ctxt_b[b][j]
                rs = npool.tile([1, 512], F32, tag="rs", name="rs")
                nc.vector.tensor_copy(rs[:], pvt[DH : DH + 1, :])
                rr = npool.tile([1, 512], F32, tag="rr", name="rr")
                nc.vector.reciprocal_approx_fast(rr[:], rs[:])
                rb = npool.tile([DH, 512], F32, tag="rb", name="rb")
                nc.gpsimd.partition_broadcast(rb[:], rr[:])
                if row0 == 0:
                    nc.vector.tensor_tensor(
                        ct[0:DH, c * 512 : (c + 1) * 512], pvt[0:DH, :], rb[:], MULT
                    )
                else:
                    ch = npool.tile([DH, 512], BF16, tag="ch", name="ch")
                    nc.vector.tensor_tensor(ch[:], pvt[0:DH, :], rb[:], MULT)
                    nc.sync.dma_start(
                        ct[row0 : row0 + DH, c * 512 : (c + 1) * 512], ch[:]
                    )

            def head_phase(b, j, p):
                h = 2 * j + p
                qtj, ktj0, ktj1 = qk_cur[(b, j)]
                ktp = ktj0 if p == 0 else ktj1
                v = v_tiles[b]
                pts = [
                    ptpool.tile([P, TB], BF16, tag=f"pt{st % 4}", name=f"pt{st}")
                    for st in range(KD)
                ]

                pvt = {}

                def pv_step(c, st):
                    if st == 0:
                        pvt[c] = pvpool.tile([DH + 1, 512], F32, tag="pv", name="pvt")
                    nc.tensor.matmul(
                        pvt[c][:],
                        v[st][:, h, :],
                        pts[st][:, c * 512 : (c + 1) * 512],
                        start=(st == 0),
                        stop=(st == KD - 1),
                    )
                    if st == KD - 1:
                        normalize(b, h, c, pvt[c])

                # 2 score-tiles per round: k=64 score matmuls grouped so the
                # PE pays the k-geometry-change penalty (~90ns) once per
                # round instead of once per score tile.
                def sc_pair(st):
                    ps = scpool.tile([P, TB], F32, tag="sc", name="scps")
                    for c in range(2):
                        nc.tensor.matmul(
                            ps[:, c * 512 : (c + 1) * 512],
                            ktp[:, st * P : (st + 1) * P],
                            qtj[:, c * 512 : (c + 1) * 512],
                            start=True,
                            stop=True,
                        )
                    nc.scalar.activation(pts[st][:], ps[:], EXPF, scale=0.125)

                for st0 in range(0, KD, 2):
                    sc_pair(st0)
                    sc_pair(st0 + 1)
                    if st0 >= 2:
                        pv_step(0, st0 - 2)
                        pv_step(1, st0 - 2)
                        pv_step(0, st0 - 1)
                        pv_step(1, st0 - 1)
                    pull_fillers(5)
                pv_step(0, KD - 2)
                pv_step(1, KD - 2)
                pull_fillers(2)
                pv_step(0, KD - 1)
                pv_step(1, KD - 1)
                pull_fillers(2)

            # ====================== schedule ======================
            # preamble: V(b0) fully, then QK(b0, j0)
            make_v_tiles(0)
            for c in range(2):
                for mt in range(KD):
                    for _ in v_unit(0, mt, c):
                        pass
            sca = scpool.tile([P, TB], F32, tag="sc", name="scpre")
            scb = scpool.tile([P, TB], F32, tag="sc", name="scpre")
            pre_slots = [
                sca[:, 0:512],
                sca[:, 512:1024],
                scb[:, 0:512],
                scb[:, 512:1024],
            ]
            for g in make_qk(0, 0, ps_slots=pre_slots):
                for _ in g:
                    pass
            make_ctxt(0)

            for b in range(NB):
                for j in range(KD):
                    # queue fillers for this j
                    if j < KD - 1:
                        filler_q.extend(make_qk(b, j + 1))
                    elif b + 1 < NB:
                        filler_q.extend(make_qk(b + 1, 0))
                    if b == 0:
                        if j == 0:
                            make_v_tiles(1)
                        filler_q.append(v_unit(1, j, 0))
                        filler_q.append(v_unit(1, j, 1))
                    else:
                        filler_q.append(out_unit(0, j, 0))
                        filler_q.append(out_unit(0, j, 1))
                        if j == KD - 1:
                            # last head-pair has no Q/K-projection fillers:
                            # feed it the first two tail chains' k0..k6
                            # (their k7 needs the final ctxt tile, deferred
                            # to the tail so it can't stall the phase).
                            for c in range(2):
                                g = out_unit(1, 0, c)
                                tail_prefix.append(g)

                                def pw(g=g):
                                    for _ in range(KD - 1):
                                        yield next(g)

                                filler_q.append(pw())
                    if b == 1 and j == 0:
                        make_ctxt(1)
                    head_phase(b, j, 0)
                    head_phase(b, j, 1)

            drain_fillers()
            # tail: output projection of the last batch.  Every chain stalls
            # on the very last normalize at its final (k=7) step, so run
            # waves of 6 chains (borrowing the now-idle score/PV PSUM banks):
            # emit all 6 chains' k0..k6 (fill), then the k7s + drains.
            units = [(mo, c) for mo in range(KD) for c in range(2)]
            if tail_prefix:
                units = units[2:]  # (mo=0, c=0/1) already prefixed as fillers

            def borrow_slots():
                sct = scpool.tile([P, TB], F32, tag="sc", name="sctail")
                return [
                    sct[:, 0:512],
                    sct[:, 512:1024],
                    pvpool.tile([P, 512], F32, tag="pv", name="pvtail")[:],
                    pvpool.tile([P, 512], F32, tag="pv", name="pvtail")[:],
                ]

            def start_group(batch, slots):
                gens = [
                    out_unit(1, mo, c, ps, drain=("act" if gi % 2 else "dve"))
                    for gi, ((mo, c), ps) in enumerate(zip(batch, slots))
                ]
                for g in gens:
                    for _ in range(KD - 1):
                        next(g, None)
                return gens

            def finish(gens):
                for g in gens:
                    for _ in g:  # k7 + bias-add + DMA
                        pass

            ga = start_group(units[0:4], borrow_slots())
            finish(tail_prefix)
            gc = start_group(units[4:6], [None, None])
            finish(ga)
            gb = start_group(units[6:10], borrow_slots())
            finish(gc)
            gd = start_group(units[10:12], [None, None])
            finish(gb)
            ge = start_group(units[12:14], borrow_slots()[:2])
            finish(gd)
            finish(ge)

    nc.compile()
    return nc


def _get_nc():
    global _NC_CACHE
    if _NC_CACHE is None:
        _NC_CACHE = build_nc()
    return _NC_CACHE


def kernel(hidden_states, Wq, bq, Wk, bk, Wv, bv, Wo, bo):
    global LAST_RESULTS
    bf = ml_dtypes.bfloat16
    hs = np.asarray(hidden_states, np.float32)
    Wq = np.asarray(Wq, np.float32)
    Wk = np.asarray(Wk, np.float32)
    Wv = np.asarray(Wv, np.float32)
    Wo = np.asarray(Wo, np.float32)
    bq = np.asarray(bq, np.float32)
    bk = np.asarray(bk, np.float32)
    bv = np.asarray(bv, np.float32)
    bo = np.asarray(bo, np.float32)

    wqt = np.ascontiguousarray(Wq.T).astype(bf)
    wkt = np.ascontiguousarray(Wk.T).astype(bf)
    wvt = np.ascontiguousarray(Wv.T).astype(bf)
    wot = np.ascontiguousarray(Wo.T).astype(bf)
    bo2 = (bo + Wo @ bv).astype(np.float32)

    bpc = hs.shape[0] // NCORES  # batches per core
    in_maps = []
    for c in range(NCORES):
        xc = hs[c * bpc : (c + 1) * bpc].reshape(bpc * TB, D)
        in_maps.append(
            {
                "xt": np.ascontiguousarray(xc.T).astype(bf),
                "wqt": wqt,
                "wkt": wkt,
                "wvt": wvt,
                "wot": wot,
                "bq": bq,
                "bk": bk,
                "bo2": bo2,
            }
        )

    nc = _get_nc()
    res = run_bass_kernel_spmd(
        nc,
        in_maps,
        core_ids=list(range(NCORES)),
        trace=TRACE,
        **TRACE_KWARGS,
    )
    LAST_RESULTS = res

    out = np.empty((hs.shape[0], TB, D), np.float32)
    for c in range(NCORES):
        ot = res.results[c]["outt"]  # [D, T]
        for b in range(bpc):
            out[c * bpc + b] = ot[:, b * TB : (b + 1) * TB].T
    return out


# revision 13
# speedup vs baseline: 1.1995x; 1.1995x over previous
"""Multi-head attention forward (B=16, S=1024, d=1024, H=16, Dh=64) on 8
Trainium2 NeuronCores, data-parallel over batch (2 batches per core).

v2: restructured for steady-state Tensor-engine cadence (~216ns/matmul,
the 512-col streaming floor at 2.4GHz).
  - Heads processed one at a time; PV chains trail scores by 2 score-tiles
    so the PE never waits on the Scalar-engine exp.
  - K stationaries zero-padded to k=128 (two banded tiles per head pair):
    all matmuls share one PE row geometry, avoiding the ~90ns
    geometry-change penalty on LDWEIGHTS.
  - PSUM: scores 2x[128,1024] (one big exp per tile), PV 2 banks
    (sequential per-(head,chunk) chains), projections 2 banks.
  - All projection work (Q/K/V/out) emitted as uniform 8-matmul chain
    "filler" units interleaved into the attention loop so the Tensor queue
    always has ready work while exp catches up; output projection of the
    last batch drains in waves of 6 chains on borrowed idle PSUM banks.
  - DMA ordered so batch-0 tokens + Wv halves land first (V-proj starts
    ~11us in); Wo lands last (first needed ~280us in).

Device kernel (per core, bf16 matmuls, fp32 accumulate):
  inputs (host-prepped): XT [d, 2048] = hidden[2c:2c+2].reshape(2048,d).T,
  WqT/WkT/WvT = W.T [in, out], WoT = Wo.T [dv, o]  (all bf16),
  bq, bk [1024] f32, bo2 = bo + Wo @ bv  (bv folded: softmax rows sum to 1).

  QT[dq,t] = WqT.T @ XT (+bq)         KT likewise
  V[t,dv]  = XT.T @ WvT               (stored head-split with a ones column)
  per (batch, head):
    scoresT[s,t] = K_h @ Q_h.T        (k=dh=64 contraction, row-group h%2)
    PT[s,t] = exp(scoresT * 0.125)    (no max-subtract: scores bounded)
    ctxT_aug[dv+1,t] = [V_h | 1].T @ PT   (row dv = softmax denominator)
    ctxT_h = ctxT_aug[:dv] * bcast(1/denominator)
  outT[o,t] = WoT.T @ ctxT (+bo2)  ->  host transposes back.
"""

import numpy as np
import ml_dtypes

import concourse.bass as bass
import concourse.mybir as mybir
import concourse.tile as tile
from concourse import bacc
from concourse.bass_utils import run_bass_kernel_spmd

P = 128
D = 1024
T = 2048  # tokens per core
TB = 1024  # tokens per batch (= S)
H = 16
DH = 64
KD = D // P  # 8 partition-tiles of the d/dv/s dims
NB = T // TB  # batches per core
NCORES = 8

BF16 = mybir.dt.bfloat16
F32 = mybir.dt.float32
EXPF = mybir.ActivationFunctionType.Exp
MULT = mybir.AluOpType.mult

# test.py hooks
TRACE = False
TRACE_KWARGS = {}
LAST_RESULTS = None

_NC_CACHE = None


def build_nc():
    nc = bacc.Bacc("TRN2", target_bir_lowering=False, debug=False, num_devices=NCORES)

    xt_d = nc.dram_tensor("xt", [D, T], BF16, kind="ExternalInput")
    wqt_d = nc.dram_tensor("wqt", [D, D], BF16, kind="ExternalInput")
    wkt_d = nc.dram_tensor("wkt", [D, D], BF16, kind="ExternalInput")
    wvt_d = nc.dram_tensor("wvt", [D, D], BF16, kind="ExternalInput")
    wot_d = nc.dram_tensor("wot", [D, D], BF16, kind="ExternalInput")
    bq_d = nc.dram_tensor("bq", [D], F32, kind="ExternalInput")
    bk_d = nc.dram_tensor("bk", [D], F32, kind="ExternalInput")
    bo2_d = nc.dram_tensor("bo2", [D], F32, kind="ExternalInput")
    outt_d = nc.dram_tensor("outt", [D, T], F32, kind="ExternalOutput")

    with tile.TileContext(nc) as tc:
        from contextlib import ExitStack

        with ExitStack() as ctx:
            wpool = ctx.enter_context(tc.tile_pool(name="w", bufs=1))
            xpool = ctx.enter_context(tc.tile_pool(name="x", bufs=1))
            qkpool = ctx.enter_context(tc.tile_pool(name="qk", bufs=2))
            vpool = ctx.enter_context(tc.tile_pool(name="v", bufs=2))
            ptpool = ctx.enter_context(tc.tile_pool(name="pt", bufs=2))
            cpool = ctx.enter_context(tc.tile_pool(name="ctx", bufs=1))
            spool = ctx.enter_context(tc.tile_pool(name="small", bufs=1))
            npool = ctx.enter_context(tc.tile_pool(name="norm", bufs=2))
            opool = ctx.enter_context(tc.tile_pool(name="out", bufs=6))
            scpool = ctx.enter_context(tc.tile_pool(name="sc", bufs=2, space="PSUM"))
            pvpool = ctx.enter_context(tc.tile_pool(name="pv", bufs=2, space="PSUM"))
            fpool = ctx.enter_context(tc.tile_pool(name="fp", bufs=2, space="PSUM"))

            # ---- global loads (ordered so V-proj can start earliest) ----
            xt = [xpool.tile([P, T], BF16, tag=f"xt{k}", name=f"xt{k}") for k in range(KD)]
            wq, wk, wv, wo = (
                [wpool.tile([P, D], BF16, tag=f"w{nm}{k}", name=f"w{nm}{k}") for k in range(KD)]
                for nm in "qkvo"
            )
            bq_sb = spool.tile([P, KD], F32, tag="bq", name="bq_sb")
            bk_sb = spool.tile([P, KD], F32, tag="bk", name="bk_sb")
            bo_sb = spool.tile([P, KD], F32, tag="bo", name="bo_sb")
            # batch-0 tokens + Wv first (V-proj of batch 0 is the first
            # compute), then Wq/Wk (Q/K proj), batch-1 tokens, Wo last.
            # The strided (descriptor-heavy) bias gathers are deferred behind
            # the V-proj-critical loads: biases are first read ~35us in.
            for k in range(KD):
                nc.sync.dma_start(xt[k][:, 0:512], xt_d[k * P : (k + 1) * P, 0:512])
                nc.sync.dma_start(wv[k][:, 0:512], wvt_d[k * P : (k + 1) * P, 0:512])
            for sb, dte in ((bq_sb, bq_d), (bk_sb, bk_d), (bo_sb, bo2_d)):
                nc.sync.dma_start(sb[:], dte.rearrange("(o p) -> p o", p=P))
            for k in range(KD):
                nc.sync.dma_start(xt[k][:, 512:TB], xt_d[k * P : (k + 1) * P, 512:TB])
            for k in range(KD):
                nc.sync.dma_start(wv[k][:, 512:D], wvt_d[k * P : (k + 1) * P, 512:D])
            for k in range(KD):
                nc.sync.dma_start(wq[k][:], wqt_d[k * P : (k + 1) * P, :])
                nc.sync.dma_start(wk[k][:], wkt_d[k * P : (k + 1) * P, :])
            for k in range(KD):
                nc.sync.dma_start(xt[k][:, TB:T], xt_d[k * P : (k + 1) * P, TB:T])
            for k in range(KD):
                nc.sync.dma_start(wo[k][:], wot_d[k * P : (k + 1) * P, :])

            # ---- state ----
            tail_prefix = []
            qk_instance = [0]
            v_tiles = {}  # b -> [v tile per mt]
            qk_cur = {}  # j-slot -> (qtj, ktj); only current+next alive
            ctxt_b = {}  # b -> [ctxt tile per m]

            # =========== filler units (uniform 8-matmul chains) ============
            def v_unit(b, mt, c):
                """V[t-tile mt, dv chunk c] for batch b -> v_tiles[b][mt]."""
                vt = v_tiles[b][mt]
                ps = fpool.tile([P, 512], F32, tag="fp", name="fpv")
                for k in range(KD):
                    yield nc.tensor.matmul(
                        ps[:],
                        xt[k][:, (b * KD + mt) * P : (b * KD + mt + 1) * P],
                        wv[k][:, c * 512 : (c + 1) * 512],
                        start=(k == 0),
                        stop=(k == KD - 1),
                    )
                nc.vector.tensor_copy(
                    vt[:, c * 8 : (c + 1) * 8, 0:DH],
                    ps.rearrange("p (h d) -> p h d", d=DH),
                )

            def qk_unit(wt, bias_sb, dests, b, j, c, ps=None):
                """One 512-token chunk of the Q or K projection for head-pair
                j.  dests: list of (tile, p_lo, p_hi) partition-row spans."""
                if ps is None:
                    ps = fpool.tile([P, 512], F32, tag="fp", name="fpq")
                for k in range(KD):
                    yield nc.tensor.matmul(
                        ps[:],
                        wt[k][:, j * P : (j + 1) * P],
                        xt[k][:, b * TB + c * 512 : b * TB + (c + 1) * 512],
                        start=(k == 0),
                        stop=(k == KD - 1),
                    )
                for dest, lo, hi in dests:
                    nc.vector.tensor_scalar_add(
                        dest[lo:hi, c * 512 : (c + 1) * 512],
                        ps[lo:hi, :],
                        bias_sb[lo:hi, j : j + 1],
                    )

            def out_unit(b, mo, c, ps=None, drain="dve"):
                """outT[o-tile mo, chunk c] for batch b."""
                ct = ctxt_b[b]
                if ps is None:
                    ps = fpool.tile([P, 512], F32, tag="fp", name="fpo")
                for k in range(KD):
                    yield nc.tensor.matmul(
                        ps[:],
                        wo[k][:, mo * P : (mo + 1) * P],
                        ct[k][:, c * 512 : (c + 1) * 512],
                        start=(k == 0),
                        stop=(k == KD - 1),
                    )
                osb = opool.tile([P, 512], F32, tag="osb", name="osb")
                if drain == "act":
                    nc.scalar.add(osb[:], ps[:], bo_sb[:, mo : mo + 1])
                else:
                    nc.vector.tensor_scalar_add(osb[:], ps[:], bo_sb[:, mo : mo + 1])
                nc.sync.dma_start(
                    outt_d[
                        mo * P : (mo + 1) * P,
                        b * TB + c * 512 : b * TB + (c + 1) * 512,
                    ],
                    osb[:],
                )

            filler_q = []  # list of active generators

            def pull_fillers(n):
                while n > 0 and filler_q:
                    g = filler_q[0]
                    try:
                        next(g)
                        n -= 1
                    except StopIteration:
                        filler_q.pop(0)

            def drain_fillers():
                while filler_q:
                    pull_fillers(1)

            def make_v_tiles(b):
                tiles = []
                for mt in range(KD):
                    vt = vpool.tile(
                        [P, H, DH + 1], BF16, tag=f"v{mt}", name=f"v{mt}"
                    )
                    nc.vector.memset(vt[:, :, DH : DH + 1], 1.0)
                    tiles.append(vt)
                v_tiles[b] = tiles

            def make_qk(b, j, ps_slots=None):
                # K is stored as two zero-padded [128, TB] tiles (head h in
                # its own 64-row band, zeros elsewhere) so score matmuls run
                # with k=128 stationary geometry — avoiding the ~90ns PE
                # row-geometry-change penalty of k=64 loads.
                qtj = qkpool.tile([P, TB], BF16, tag="qtj", name="qtj")
                ktj0 = qkpool.tile([P, TB], BF16, tag="ktj0", name="ktj0")
                ktj1 = qkpool.tile([P, TB], BF16, tag="ktj1", name="ktj1")
                # zero bands only for the first instance in each rotating
                # buffer: later instances reuse the same SBUF bytes and only
                # rows 0:DH / DH:P are ever rewritten, so zeros persist.
                if qk_instance[0] < 2:
                    nc.vector.memset(ktj0[DH:P, :], 0.0)
                    nc.vector.memset(ktj1[0:DH, :], 0.0)
                qk_instance[0] += 1
                qk_cur[(b, j)] = (qtj, ktj0, ktj1)
                units = []
                slots = ps_slots or [None] * 4
                for c in range(2):
                    units.append(
                        qk_unit(wq, bq_sb, [(qtj, 0, P)], b, j, c, ps=slots[c])
                    )
                for c in range(2):
                    units.append(
                        qk_unit(
                            wk,
                            bk_sb,
                            [(ktj0, 0, DH), (ktj1, DH, P)],
                            b,
                            j,
                            c,
                            ps=slots[2 + c],
                        )
                    )
                return units

            def make_ctxt(b):
                if b == 0:
                    tiles = [
                        cpool.tile([P, TB], BF16, tag=f"ctxt{m}", name=f"ctxt{m}")
                        for m in range(KD)
                    ]
                else:
                    # reuse wv's SBUF slots: wv is fully consumed by the time
                    # batch-1 context tiles are first written (shapes match)
                    tiles = [
                        wpool.tile([P, TB], BF16, tag=f"wv{m}", name=f"ctxb1_{m}")
                        for m in range(KD)
                    ]
                ctxt_b[b] = tiles

            # ================= attention head phase =================
            def normalize(b, h, c, pvt):
                j, row0 = h // 2, (h % 2) * DH
                ct = ctxt_b[b][j]
                rs = npool.tile([1, 512], F32, tag="rs", name="rs")
                nc.vector.tensor_copy(rs[:], pvt[DH : DH + 1, :])
                rr = npool.tile([1, 512], F32, tag="rr", name="rr")
                nc.vector.reciprocal_approx_fast(rr[:], rs[:])
                rb = npool.tile([DH, 512], F32, tag="rb", name="rb")
                nc.gpsimd.partition_broadcast(rb[:], rr[:])
                if row0 == 0:
                    nc.vector.tensor_tensor(
                        ct[0:DH, c * 512 : (c + 1) * 512], pvt[0:DH, :], rb[:], MULT
                    )
                else:
                    ch = npool.tile([DH, 512], BF16, tag="ch", name="ch")
                    nc.vector.tensor_tensor(ch[:], pvt[0:DH, :], rb[:], MULT)
                    nc.sync.dma_start(
                        ct[row0 : row0 + DH, c * 512 : (c + 1) * 512], ch[:]
                    )

            def head_phase(b, j, p):
                h = 2 * j + p
                qtj, ktj0, ktj1 = qk_cur[(b, j)]
                ktp = ktj0 if p == 0 else ktj1
                v = v_tiles[b]
                pts = [
                    ptpool.tile([P, TB], BF16, tag=f"pt{st % 4}", name=f"pt{st}")
                    for st in range(KD)
                ]

                pvt = {}

                def pv_step(c, st):
                    if st == 0:
                        pvt[c] = pvpool.tile([DH + 1, 512], F32, tag="pv", name="pvt")
                    nc.tensor.matmul(
                        pvt[c][:],
                        v[st][:, h, :],
                        pts[st][:, c * 512 : (c + 1) * 512],
                        start=(st == 0),
                        stop=(st == KD - 1),
                    )
                    if st == KD - 1:
                        normalize(b, h, c, pvt[c])

                # 2 score-tiles per round: k=64 score matmuls grouped so the
                # PE pays the k-geometry-change penalty (~90ns) once per
                # round instead of once per score tile.
                def sc_pair(st):
                    ps = scpool.tile([P, TB], F32, tag="sc", name="scps")
                    for c in range(2):
                        nc.tensor.matmul(
                            ps[:, c * 512 : (c + 1) * 512],
                            ktp[:, st * P : (st + 1) * P],
                            qtj[:, c * 512 : (c + 1) * 512],
                            start=True,
                            stop=True,
                        )
                    nc.scalar.activation(pts[st][:], ps[:], EXPF, scale=0.125)

                for st0 in range(0, KD, 2):
                    sc_pair(st0)
                    sc_pair(st0 + 1)
                    if st0 >= 2:
                        pv_step(0, st0 - 2)
                        pv_step(1, st0 - 2)
                        pv_step(0, st0 - 1)
                        pv_step(1, st0 - 1)
                    pull_fillers(5)
                pv_step(0, KD - 2)
                pv_step(1, KD - 2)
                pull_fillers(2)
                pv_step(0, KD - 1)
                pv_step(1, KD - 1)
                pull_fillers(2)

            # ====================== schedule ======================
            # preamble: V(b0) fully, then QK(b0, j0)
            make_v_tiles(0)
            for c in range(2):
                for mt in range(KD):
                    for _ in v_unit(0, mt, c):
                        pass
            sca = scpool.tile([P, TB], F32, tag="sc", name="scpre")
            scb = scpool.tile([P, TB], F32, tag="sc", name="scpre")
            pre_slots = [
                sca[:, 0:512],
                sca[:, 512:1024],
                scb[:, 0:512],
                scb[:, 512:1024],
            ]
            for g in make_qk(0, 0, ps_slots=pre_slots):
                for _ in g:
                    pass
            make_ctxt(0)

            for b in range(NB):
                for j in range(KD):
                    # queue fillers for this j
                    if j < KD - 1:
                        filler_q.extend(make_qk(b, j + 1))
                    elif b + 1 < NB:
                        filler_q.extend(make_qk(b + 1, 0))
                    if b == 0:
                        if j == 0:
                            make_v_tiles(1)
                        filler_q.append(v_unit(1, j, 0))
                        filler_q.append(v_unit(1, j, 1))
                    else:
                        filler_q.append(out_unit(0, j, 0))
                        filler_q.append(out_unit(0, j, 1))
                        if j == KD - 1:
                            # last head-pair has no Q/K-projection fillers:
                            # feed it the first two tail chains' k0..k6
                            # (their k7 needs the final ctxt tile, deferred
                            # to the tail so it can't stall the phase).
                            for c in range(2):
                                g = out_unit(1, 0, c)
                                tail_prefix.append(g)

                                def pw(g=g):
                                    for _ in range(KD - 1):
                                        yield next(g)

                                filler_q.append(pw())
                    if b == 1 and j == 0:
                        make_ctxt(1)
                    head_phase(b, j, 0)
                    head_phase(b, j, 1)

            drain_fillers()
            # tail: output projection of the last batch.  Every chain stalls
            # on the very last normalize at its final (k=7) step, so run
            # waves of 6 chains (borrowing the now-idle score/PV PSUM banks):
            # emit all 6 chains' k0..k6 (fill), then the k7s + drains.
            units = [(mo, c) for mo in range(KD) for c in range(2)]
            if tail_prefix:
                units = units[2:]  # (mo=0, c=0/1) already prefixed as fillers

            def borrow_slots():
                sct = scpool.tile([P, TB], F32, tag="sc", name="sctail")
                return [
                    sct[:, 0:512],
                    sct[:, 512:1024],
                    pvpool.tile([P, 512], F32, tag="pv", name="pvtail")[:],
                    pvpool.tile([P, 512], F32, tag="pv", name="pvtail")[:],
                ]

            def start_group(batch, slots):
                gens = [
                    out_unit(1, mo, c, ps, drain=("act" if gi % 2 else "dve"))
                    for gi, ((mo, c), ps) in enumerate(zip(batch, slots))
                ]
                for g in gens:
                    for _ in range(KD - 1):
                        next(g, None)
                return gens

            def finish(gens):
                for g in gens:
                    for _ in g:  # k7 + bias-add + DMA
                        pass

            ga = start_group(units[0:4], borrow_slots())
            finish(tail_prefix)
            gc = start_group(units[4:6], [None, None])
            finish(ga)
            gb = start_group(units[6:10], borrow_slots())
            finish(gc)
            gd = start_group(units[10:12], [None, None])
            finish(gb)
            ge = start_group(units[12:14], borrow_slots()[:2])
            finish(gd)
            finish(ge)

    nc.compile()
    return nc


def _get_nc():
    global _NC_CACHE
    if _NC_CACHE is None:
        _NC_CACHE = build_nc()
    return _NC_CACHE


def kernel(hidden_states, Wq, bq, Wk, bk, Wv, bv, Wo, bo):
    global LAST_RESULTS
    bf = ml_dtypes.bfloat16
    hs = np.asarray(hidden_states, np.float32)
    Wq = np.asarray(Wq, np.float32)
    Wk = np.asarray(Wk, np.float32)
    Wv = np.asarray(Wv, np.float32)
    Wo = np.asarray(Wo, np.float32)
    bq = np.asarray(bq, np.float32)
    bk = np.asarray(bk, np.float32)
    bv = np.asarray(bv, np.float32)
    bo = np.asarray(bo, np.float32)

    wqt = np.ascontiguousarray(Wq.T).astype(bf)
    wkt = np.ascontiguousarray(Wk.T).astype(bf)
    wvt = np.ascontiguousarray(Wv.T).astype(bf)
    wot = np.ascontiguousarray(Wo.T).astype(bf)
    bo2 = (bo + Wo @ bv).astype(np.float32)

    bpc = hs.shape[0] // NCORES  # batches per core
    in_maps = []
    for c in range(NCORES):
        xc = hs[c * bpc : (c + 1) * bpc].reshape(bpc * TB, D)
        in_maps.append(
            {
                "xt": np.ascontiguousarray(xc.T).astype(bf),
                "wqt": wqt,
                "wkt": wkt,
                "wvt": wvt,
                "wot": wot,
                "bq": bq,
                "bk": bk,
                "bo2": bo2,
            }
        )

    nc = _get_nc()
    res = run_bass_kernel_spmd(
        nc,
        in_maps,
        core_ids=list(range(NCORES)),
        trace=TRACE,
        **TRACE_KWARGS,
    )
    LAST_RESULTS = res

    out = np.empty((hs.shape[0], TB, D), np.float32)
    for c in range(NCORES):
        ot = res.results[c]["outt"]  # [D, T]
        for b in range(bpc):
            out[c * bpc + b] = ot[:, b * TB : (b + 1) * TB].T
    return out
